# revision 31
# baseline (speedup 1.0000x reference)
"""Multi-head contextual biased attention on 8 Trainium2 NeuronCores.

Sharding: data-parallel over batch (B=2) x tensor-parallel over heads
(16 heads -> 4 per core).  Each core computes Q/K/V projections for its
4 heads, attention with the periodic ALiBi-style bias applied as a
precomputed multiplicative table, and a partial output projection; the
host sums the 4 partial projections per batch element and adds bo.

v3 layout notes:
  - the attention loop is ACT-paced (one [128,1024] f32->bf16 Exp per
    jt is the hard floor: 128 instrs x ~1.15us).  Everything else is
    shaped to hide underneath that.
  - per jt one shared score tile sp[128 j, 1024] holds BOTH heads of a
    pair: head A's i-chunk in cols 0:512 (computed on PE row-tile
    (0,0) from kT/qT partitions 0:64) and head B's in cols 512:1024
    (row-tile (64,0)).  The two QK matmuls hit different row tiles AND
    different PSUM banks, so they stream concurrently; one Exp covers
    both heads.
  - pv[65, 1024] likewise holds both heads ([pv_A | pv_B]); the ones
    column of v gives both softmax denominators at partition 64.
    PSUM: sp ring-2 (4 banks) + pv ring-2 (4 banks) = 8.
  - projections run k-outer (accumulators live across the k DMA
    stream) so the PE starts ~2us after the first xt chunk lands.
  - exp(bias) tables are applied multiplicatively on the DVE in bf16
    2x mode; o2 normalization multiplies run on gpsimd to keep the DVE
    under the ACT pace.
"""

import numpy as np
import ml_dtypes
from contextlib import ExitStack

import concourse.bass as bass
import concourse.tile as tile
from concourse import bacc, mybir
from concourse.alu_op_type import AluOpType
from concourse.bass_utils import run_bass_kernel_spmd

bf16 = ml_dtypes.bfloat16
fp8 = ml_dtypes.float8_e4m3fn
F32 = mybir.dt.float32
BF16 = mybir.dt.bfloat16
FP8 = mybir.dt.float8e4
DR = mybir.MatmulPerfMode.DoubleRow
Exp = mybir.ActivationFunctionType.Exp

# fp8 V-projection scaling: wv is multiplied by WS on the host so the
# uniform(-1/32, 1/32) entries use fp8e4's normal range; v's WS-fold
# inflation is divided back out of Wo on the host.  Q/K stay bf16 (their
# quantization error enters exp() and is too hot for the tolerance).
WS = 32.0

B, T, D = 2, 2048, 1024
NH, DH = 16, 64          # global heads, head dim
HL = 4                   # heads per core
KC = D // 128            # contraction chunks
PERIOD = 30
T0 = 2049                # odd skew origin (odd => step -1 APs stay aligned)
EBL = 3972               # skew table length
IQ = 512                 # per-head i-chunk per block
LAG = 3                  # PV emission lag in jt


def _build_kernel(ctx, tc, y_d, xt_d, ct_d, wq_d, wk_d, wv_d, wo_d, eb_d):
    nc = tc.nc

    const = ctx.enter_context(tc.tile_pool(name="const", bufs=1))
    data = ctx.enter_context(tc.tile_pool(name="data", bufs=1))

    # DMA issue order matters: single queue drains in order.
    wq_sb = const.tile([128, KC, 256], BF16)
    nc.sync.dma_start(wq_sb[:], wq_d[:])
    xt_sb = data.tile([128, KC, T], BF16)
    for k in range(KC):
        nc.sync.dma_start(xt_sb[:, k, :], xt_d[:, k, :])
    wk_sb = const.tile([128, KC, 256], BF16)
    nc.sync.dma_start(wk_sb[:], wk_d[:])
    wv_sb = const.tile([128, KC, 256], BF16)
    nc.sync.dma_start(wv_sb[:], wv_d[:])
    ct_sb = data.tile([128, KC, T], BF16)
    for k in range(KC):
        nc.scalar.dma_start(ct_sb[:, k, :], ct_d[:, k, :])
    eb_sb = const.tile([128, HL, EBL], BF16)
    for h in range(2):
        nc.sync.dma_start(eb_sb[:, h, :], eb_d[:, h, :])
    wo_sb = const.tile([128, 2, D], BF16)
    nc.sync.dma_start(wo_sb[:], wo_d[:])
    for h in range(2, HL):
        nc.sync.dma_start(eb_sb[:, h, :], eb_d[:, h, :])

    qT_sb = data.tile([128, 2, T], BF16)
    kT_sb = data.tile([128, 2, T], BF16)
    o2_sb = data.tile([128, 2, T], BF16)
    v_sb = data.tile([128, HL, 16, 80], BF16)   # 64 d + ones col at 64
    nc.vector.memset(v_sb[:, :, :, 64:65], 1.0)

    # ---- Phase 1: fp8 DoubleRow projections, k-outer over DMA chunks ----
    KP = KC // 2  # k-subtile pairs per accumulation
    with tc.tile_pool(name="pps", bufs=8, space="PSUM") as pps:
        qps = {}
        for it in range(4):
            qps[it] = pps.tile([128, 512], F32, tag="mm", name=f"qps_{it}")
        for k in range(KC):
            for it in range(4):
                nc.tensor.matmul(qps[it][:],
                                 lhsT=wq_sb[:, k, 0:128],
                                 rhs=xt_sb[:, k, it * 512:(it + 1) * 512],
                                 start=(k == 0), stop=(k == KC - 1))
        for it in range(4):
            nc.scalar.copy(qT_sb[:, 0, it * 512:(it + 1) * 512], qps[it][:])
        kps = {}
        for it in range(4):
            kps[it] = pps.tile([128, 512], F32, tag="mm", name=f"kps_{it}")
        for k in range(KC):
            for it in range(4):
                nc.tensor.matmul(kps[it][:],
                                 lhsT=wk_sb[:, k, 0:128],
                                 rhs=ct_sb[:, k, it * 512:(it + 1) * 512],
                                 start=(k == 0), stop=(k == KC - 1))
        for it in range(4):
            nc.scalar.copy(kT_sb[:, 0, it * 512:(it + 1) * 512], kps[it][:])
        for jtg in range(2):
            vps = {}
            for j4 in range(4):
                vps[j4] = pps.tile([128, 512], F32, tag="mm",
                                   name=f"vps_{jtg}_{j4}")
            for k in range(KC):
                for j4 in range(4):
                    jt = jtg * 4 + j4
                    nc.tensor.matmul(vps[j4][:, 0:256],
                                     lhsT=ct_sb[:, k, jt * 128:(jt + 1) * 128],
                                     rhs=wv_sb[:, k, :],
                                     start=(k == 0), stop=(k == KC - 1))
            for j4 in range(4):
                jt = jtg * 4 + j4
                nc.vector.tensor_copy(
                    v_sb[:, :, jt, 0:64],
                    vps[j4][:, 0:256].rearrange("p (h d) -> p h d", h=HL))

    # ---- Phase 2: attention, 8 blocks of (head-pair m, i-chunk iq) ----
    # deferred m=1 q/k projections run as background closures in the PE
    # idle of the first blocks, using the 2 PSUM banks freed by pv ring-1.
    def mk_bgproj(w_sb, src_sb, dst_sb, it, nm):
        def go(pool):
            ps = pool.tile([128, 512], F32, tag="bg", name=f"bg{nm}_{it}")
            for k in range(KC):
                nc.tensor.matmul(ps[:], lhsT=w_sb[:, k, 128:256],
                                 rhs=src_sb[:, k, it * 512:(it + 1) * 512],
                                 start=(k == 0), stop=(k == KC - 1))
            nc.vector.tensor_copy(dst_sb[:, 1, it * 512:(it + 1) * 512], ps[:])
        return go

    def mk_bgv(jt):
        def go(pool):
            ps = pool.tile([128, 512], F32, tag="bg", name=f"bgv_{jt}")
            for k in range(KC):
                nc.tensor.matmul(ps[:, 0:256],
                                 lhsT=ct_sb[:, k, jt * 128:(jt + 1) * 128],
                                 rhs=wv_sb[:, k, :],
                                 start=(k == 0), stop=(k == KC - 1))
            nc.vector.tensor_copy(
                v_sb[:, :, jt, 0:64],
                ps[:, 0:256].rearrange("p (h d) -> p h d", h=HL))
        return go

    bg = [mk_bgv(jt) for jt in range(8, 16)]
    bg += [mk_bgproj(wq_sb, xt_sb, qT_sb, it, "q") for it in range(4)]
    bg += [mk_bgproj(wk_sb, ct_sb, kT_sb, it, "k") for it in range(4)]

    with tc.tile_pool(name="sps", bufs=2, space="PSUM") as sps, \
         tc.tile_pool(name="pvs", bufs=1, space="PSUM") as pvs, \
         tc.tile_pool(name="bgp", bufs=2, space="PSUM") as bgp, \
         tc.tile_pool(name="ptp", bufs=6) as ptp, \
         tc.tile_pool(name="nrm", bufs=2) as nrm:
        for m in range(2):
            hA, hB = 2 * m, 2 * m + 1
            for iq in range(4):
                i0 = iq * IQ
                pv = pvs.tile([65, 1024], F32, tag="pv", name=f"pv_{m}_{iq}")

                def emit_pv(jt, pt):
                    first, last = jt == 0, jt == 15
                    nc.tensor.matmul(pv[:, 0:512], lhsT=v_sb[:, hA, jt, 0:65],
                                     rhs=pt[:, 0:512], start=first, stop=last)
                    nc.tensor.matmul(pv[:, 512:1024], lhsT=v_sb[:, hB, jt, 0:65],
                                     rhs=pt[:, 512:1024], start=first, stop=last)

                pending = []
                for jt in range(16):
                    sp = sps.tile([128, 1024], F32, tag="sp",
                                  name=f"sp_{m}_{iq}_{jt}")
                    # two heads on separate row tiles + separate banks
                    nc.tensor.matmul(sp[:, 0:512],
                                     lhsT=kT_sb[0:64, m, jt * 128:(jt + 1) * 128],
                                     rhs=qT_sb[0:64, m, i0:i0 + IQ],
                                     start=True, stop=True)
                    nc.tensor.matmul(sp[:, 512:1024],
                                     lhsT=kT_sb[64:128, m, jt * 128:(jt + 1) * 128],
                                     rhs=qT_sb[64:128, m, i0:i0 + IQ],
                                     start=True, stop=True)
                    pt = ptp.tile([128, 1024], BF16, tag="pt",
                                  name=f"pt_{m}_{iq}_{jt}")
                    nc.scalar.activation(pt[:], sp[:], Exp)
                    idx0 = T0 + jt * 128
                    nc.vector.tensor_mul(pt[:, 0:512], pt[:, 0:512],
                                         eb_sb[:, hA, idx0 - i0:idx0 - i0 - IQ:-1])
                    nc.vector.tensor_mul(pt[:, 512:1024], pt[:, 512:1024],
                                         eb_sb[:, hB, idx0 - i0:idx0 - i0 - IQ:-1])
                    pending.append((jt, pt))
                    if len(pending) > LAG:
                        emit_pv(*pending.pop(0))
                    if bg:
                        bg.pop(0)(bgp)
                for args in pending:
                    emit_pv(*args)

                # normalization: both heads' denominators sit in pv row 64
                pvf = nrm.tile([65, 1024], F32, tag="pvf", name=f"pvf_{m}_{iq}")
                nc.vector.tensor_copy(pvf[:], pv[:])
                rsq = nrm.tile([128, 8], F32, tag="rsq", name=f"rsq_{m}_{iq}")
                nc.sync.dma_start(rsq[:], pvf[64:65, :])
                rsr = nrm.tile([128, 8], F32, tag="rsr", name=f"rsr_{m}_{iq}")
                nc.vector.reciprocal(rsr[:], rsq[:])
                rsf = nrm.tile([1, 1024], F32, tag="rsf", name=f"rsf_{m}_{iq}")
                nc.sync.dma_start(rsf[:], rsr[:])
                rsb = nrm.tile([64, 1024], F32, tag="rsb", name=f"rsb_{m}_{iq}")
                nc.gpsimd.partition_broadcast(rsb[:], rsf[:], channels=64)
                nc.vector.tensor_mul(o2_sb[0:64, m, i0:i0 + IQ],
                                     pvf[0:64, 0:512], rsb[:, 0:512])
                otmp = nrm.tile([64, 512], BF16, tag="otmp", name=f"otmp_{m}_{iq}")
                nc.vector.tensor_mul(otmp[:], pvf[0:64, 512:1024],
                                     rsb[:, 512:1024])
                nc.sync.dma_start(o2_sb[64:128, m, i0:i0 + IQ], otmp[:])

    # ---- Phase 3: output projection (partial; host sums head-groups) ----
    # yt batches both mt halves -> one DMA per ic, issued off the sync
    # queue (alternating scalar/vector) to dodge per-issue serialization.
    with tc.tile_pool(name="yps", bufs=4, space="PSUM") as yps, \
         tc.tile_pool(name="yo", bufs=3) as yo:
        for ic in range(16):
            yt = yo.tile([128, 1024], BF16, tag="yt", name=f"yt_{ic}")
            for mt in range(2):
                ps = yps.tile([128, 512], F32, tag="y", name=f"yps_{ic}_{mt}")
                for m in range(2):
                    nc.tensor.matmul(ps[:], lhsT=o2_sb[:, m, ic * 128:(ic + 1) * 128],
                                     rhs=wo_sb[:, m, mt * 512:(mt + 1) * 512],
                                     start=(m == 0), stop=(m == 1))
                if mt == 0:
                    nc.scalar.copy(yt[:, 0:512], ps[:])
                else:
                    nc.vector.tensor_copy(yt[:, 512:1024], ps[:])
            eng = nc.scalar if ic % 2 == 0 else nc.sync
            eng.dma_start(y_d[ic * 128:(ic + 1) * 128, :], yt[:])


_NC = None


def build_nc():
    global _NC
    if _NC is not None:
        return _NC
    nc = bacc.Bacc("TRN2", target_bir_lowering=False, debug=False, num_devices=8)
    xt_d = nc.dram_tensor("xt", [128, KC, T], BF16, kind="ExternalInput").ap()
    ct_d = nc.dram_tensor("ct", [128, KC, T], BF16, kind="ExternalInput").ap()
    wq_d = nc.dram_tensor("wq", [128, KC, 256], BF16, kind="ExternalInput").ap()
    wk_d = nc.dram_tensor("wk", [128, KC, 256], BF16, kind="ExternalInput").ap()
    wv_d = nc.dram_tensor("wv", [128, KC, 256], BF16, kind="ExternalInput").ap()
    wo_d = nc.dram_tensor("wo", [128, 2, D], BF16, kind="ExternalInput").ap()
    eb_d = nc.dram_tensor("eb", [128, HL, EBL], BF16, kind="ExternalInput").ap()
    y_d = nc.dram_tensor("y", [T, D], BF16, kind="ExternalOutput").ap()

    with tile.TileContext(nc) as tc, ExitStack() as ctx:
        _build_kernel(ctx, tc, y_d, xt_d, ct_d, wq_d, wk_d, wv_d, wo_d, eb_d)
    nc.compile()
    _NC = nc
    return nc


def _to_chunked(mat_t, cols, dtype):
    """[D, cols] -> [128, KC, cols] with partition dim first."""
    return np.ascontiguousarray(
        mat_t.reshape(KC, 128, cols).transpose(1, 0, 2)).astype(dtype)


def make_in_maps(x, context, Wq, Wk, Wv, Wo):
    scale = np.float32(1.0 / np.sqrt(DH))
    p = np.arange(128, dtype=np.int64)[:, None]
    t = np.arange(EBL, dtype=np.int64)[None, :]
    dist = np.abs(p + t - T0) // PERIOD          # [128, EBL]
    ws = np.float32(WS)
    in_maps = []
    for c in range(8):
        b = c // 4
        h0 = (c % 4) * HL
        rows = slice(h0 * DH, (h0 + HL) * DH)
        ct_t = np.ascontiguousarray(
            context[b].T.reshape(KC, 128, T).transpose(1, 0, 2))
        xt = np.ascontiguousarray(
            x[b].T.reshape(KC, 128, T).transpose(1, 0, 2)).astype(bf16)
        ct = ct_t.astype(bf16)
        wq = _to_chunked(np.ascontiguousarray((Wq[rows] * scale).T), 256, bf16)
        wk = _to_chunked(np.ascontiguousarray(Wk[rows].T), 256, bf16)
        wv = _to_chunked(np.ascontiguousarray(Wv[rows].T), 256, bf16)
        wo = np.ascontiguousarray(
            Wo[:, rows].T.reshape(2, 128, D).transpose(1, 0, 2)).astype(bf16)
        eb = np.empty((128, HL, EBL), dtype=bf16)
        for hl in range(HL):
            hs = 2.0 ** (-(h0 + hl + 1))
            eb[:, hl, :] = np.exp(-hs * dist).astype(bf16)
        in_maps.append({"xt": xt, "ct": ct, "wq": wq, "wk": wk,
                        "wv": wv, "wo": wo, "eb": np.ascontiguousarray(eb)})
    return in_maps


def kernel(x, context, Wq, Wk, Wv, Wo, bo, _collect=None):
    x = np.asarray(x, dtype=np.float32)
    context = np.asarray(context, dtype=np.float32)
    Wq = np.asarray(Wq, dtype=np.float32)
    Wk = np.asarray(Wk, dtype=np.float32)
    Wv = np.asarray(Wv, dtype=np.float32)
    Wo = np.asarray(Wo, dtype=np.float32)
    bo = np.asarray(bo, dtype=np.float32)

    nc = build_nc()
    in_maps = make_in_maps(x, context, Wq, Wk, Wv, Wo)
    res = run_bass_kernel_spmd(nc, in_maps, list(range(8)))
    if _collect is not None:
        _collect.append(res)

    out = np.empty((B, T, D), dtype=np.float32)
    for b in range(2):
        acc = res.results[4 * b]["y"].astype(np.float32)
        for c in range(4 * b + 1, 4 * b + 4):
            acc = acc + res.results[c]["y"].astype(np.float32)
        out[b] = acc + bo[None, :]
    return out


# revision 32
# speedup vs baseline: 1.0098x; 1.0098x over previous
"""Multi-head contextual biased attention on 8 Trainium2 NeuronCores.

Sharding: data-parallel over batch (B=2) x tensor-parallel over heads
(16 heads -> 4 per core).  Each core computes Q/K/V projections for its
4 heads, attention with the periodic ALiBi-style bias applied as a
precomputed multiplicative table, and a partial output projection; the
host sums the 4 partial projections per batch element and adds bo.

v3 layout notes:
  - the attention loop is ACT-paced (one [128,1024] f32->bf16 Exp per
    jt is the hard floor: 128 instrs x ~1.15us).  Everything else is
    shaped to hide underneath that.
  - per jt one shared score tile sp[128 j, 1024] holds BOTH heads of a
    pair: head A's i-chunk in cols 0:512 (computed on PE row-tile
    (0,0) from kT/qT partitions 0:64) and head B's in cols 512:1024
    (row-tile (64,0)).  The two QK matmuls hit different row tiles AND
    different PSUM banks, so they stream concurrently; one Exp covers
    both heads.
  - pv[65, 1024] likewise holds both heads ([pv_A | pv_B]); the ones
    column of v gives both softmax denominators at partition 64.
    PSUM: sp ring-2 (4 banks) + pv ring-2 (4 banks) = 8.
  - projections run k-outer (accumulators live across the k DMA
    stream) so the PE starts ~2us after the first xt chunk lands.
  - exp(bias) tables are applied multiplicatively on the DVE in bf16
    2x mode; o2 normalization multiplies run on gpsimd to keep the DVE
    under the ACT pace.
"""

import numpy as np
import ml_dtypes
from contextlib import ExitStack

import concourse.bass as bass
import concourse.tile as tile
from concourse import bacc, mybir
from concourse.alu_op_type import AluOpType
from concourse.bass_utils import run_bass_kernel_spmd

bf16 = ml_dtypes.bfloat16
fp8 = ml_dtypes.float8_e4m3fn
F32 = mybir.dt.float32
BF16 = mybir.dt.bfloat16
FP8 = mybir.dt.float8e4
DR = mybir.MatmulPerfMode.DoubleRow
Exp = mybir.ActivationFunctionType.Exp

# fp8 V-projection scaling: wv is multiplied by WS on the host so the
# uniform(-1/32, 1/32) entries use fp8e4's normal range; v's WS-fold
# inflation is divided back out of Wo on the host.  Q/K stay bf16 (their
# quantization error enters exp() and is too hot for the tolerance).
WS = 32.0

B, T, D = 2, 2048, 1024
NH, DH = 16, 64          # global heads, head dim
HL = 4                   # heads per core
KC = D // 128            # contraction chunks
PERIOD = 30
T0 = 2049                # odd skew origin (odd => step -1 APs stay aligned)
EBL = 3972               # skew table length
IQ = 512                 # per-head i-chunk per block
LAG = 3                  # PV emission lag in jt


def _build_kernel(ctx, tc, y_d, xt_d, ct_d, wq_d, wk_d, wv_d, wo_d, eb_d):
    nc = tc.nc

    const = ctx.enter_context(tc.tile_pool(name="const", bufs=1))
    data = ctx.enter_context(tc.tile_pool(name="data", bufs=1))

    # DMA issue order matters: single queue drains in order.
    wq_sb = const.tile([128, KC, 256], BF16)
    nc.sync.dma_start(wq_sb[:], wq_d[:])
    xt_sb = data.tile([128, KC, T], BF16)
    for k in range(KC):
        nc.sync.dma_start(xt_sb[:, k, :], xt_d[:, k, :])
    wk_sb = const.tile([128, KC, 256], BF16)
    nc.sync.dma_start(wk_sb[:], wk_d[:])
    wv_sb = const.tile([128, KC, 256], BF16)
    nc.sync.dma_start(wv_sb[:], wv_d[:])
    ct_sb = data.tile([128, KC, T], BF16)
    for k in range(KC):
        nc.scalar.dma_start(ct_sb[:, k, :], ct_d[:, k, :])
    eb_sb = const.tile([128, HL, EBL], BF16)
    for h in range(2):
        nc.sync.dma_start(eb_sb[:, h, :], eb_d[:, h, :])
    wo_sb = const.tile([128, 2, D], BF16)
    nc.sync.dma_start(wo_sb[:], wo_d[:])
    for h in range(2, HL):
        nc.sync.dma_start(eb_sb[:, h, :], eb_d[:, h, :])

    qT_sb = data.tile([128, 2, T], BF16)
    kT_sb = data.tile([128, 2, T], BF16)
    o2_sb = data.tile([128, 2, T], BF16)
    v_sb = data.tile([128, HL, 16, 80], BF16)   # 64 d + ones col at 64
    nc.vector.memset(v_sb[:, :, :, 64:65], 1.0)

    # ---- Phase 1: fp8 DoubleRow projections, k-outer over DMA chunks ----
    KP = KC // 2  # k-subtile pairs per accumulation
    with tc.tile_pool(name="pps", bufs=8, space="PSUM") as pps:
        qps = {}
        for it in range(4):
            qps[it] = pps.tile([128, 512], F32, tag="mm", name=f"qps_{it}")
        for k in range(KC):
            for it in range(4):
                nc.tensor.matmul(qps[it][:],
                                 lhsT=wq_sb[:, k, 0:128],
                                 rhs=xt_sb[:, k, it * 512:(it + 1) * 512],
                                 start=(k == 0), stop=(k == KC - 1))
        for it in range(4):
            nc.scalar.copy(qT_sb[:, 0, it * 512:(it + 1) * 512], qps[it][:])
        kps = {}
        for it in range(4):
            kps[it] = pps.tile([128, 512], F32, tag="mm", name=f"kps_{it}")
        for k in range(KC):
            for it in range(4):
                nc.tensor.matmul(kps[it][:],
                                 lhsT=wk_sb[:, k, 0:128],
                                 rhs=ct_sb[:, k, it * 512:(it + 1) * 512],
                                 start=(k == 0), stop=(k == KC - 1))
        for it in range(4):
            nc.scalar.copy(kT_sb[:, 0, it * 512:(it + 1) * 512], kps[it][:])

    # ---- Phase 2: attention, 8 blocks of (head-pair m, i-chunk iq) ----
    # deferred m=1 q/k projections run as background closures in the PE
    # idle of the first blocks, using the 2 PSUM banks freed by pv ring-1.
    def mk_bgproj(w_sb, src_sb, dst_sb, it, nm):
        def go(pool):
            ps = pool.tile([128, 512], F32, tag="bg", name=f"bg{nm}_{it}")
            for k in range(KC):
                nc.tensor.matmul(ps[:], lhsT=w_sb[:, k, 128:256],
                                 rhs=src_sb[:, k, it * 512:(it + 1) * 512],
                                 start=(k == 0), stop=(k == KC - 1))
            nc.vector.tensor_copy(dst_sb[:, 1, it * 512:(it + 1) * 512], ps[:])
        return go

    def mk_bgv(jt):
        def go(pool):
            ps = pool.tile([128, 512], F32, tag="bg", name=f"bgv_{jt}")
            for k in range(KC):
                nc.tensor.matmul(ps[:, 0:256],
                                 lhsT=ct_sb[:, k, jt * 128:(jt + 1) * 128],
                                 rhs=wv_sb[:, k, :],
                                 start=(k == 0), stop=(k == KC - 1))
            nc.vector.tensor_copy(
                v_sb[:, :, jt, 0:64],
                ps[:, 0:256].rearrange("p (h d) -> p h d", h=HL))
        return go

    bg_fast = [mk_bgv(jt) for jt in range(16)]
    bg_slow = [mk_bgproj(wq_sb, xt_sb, qT_sb, it, "q") for it in range(4)]
    bg_slow += [mk_bgproj(wk_sb, ct_sb, kT_sb, it, "k") for it in range(4)]

    with tc.tile_pool(name="sps", bufs=2, space="PSUM") as sps, \
         tc.tile_pool(name="pvs", bufs=1, space="PSUM") as pvs, \
         tc.tile_pool(name="bgp", bufs=2, space="PSUM") as bgp, \
         tc.tile_pool(name="ptp", bufs=6) as ptp, \
         tc.tile_pool(name="nrm", bufs=2) as nrm:
        for m in range(2):
            hA, hB = 2 * m, 2 * m + 1
            for iq in range(4):
                i0 = iq * IQ
                pv = pvs.tile([65, 1024], F32, tag="pv", name=f"pv_{m}_{iq}")

                def emit_pv(jt, pt):
                    first, last = jt == 0, jt == 15
                    nc.tensor.matmul(pv[:, 0:512], lhsT=v_sb[:, hA, jt, 0:65],
                                     rhs=pt[:, 0:512], start=first, stop=last)
                    nc.tensor.matmul(pv[:, 512:1024], lhsT=v_sb[:, hB, jt, 0:65],
                                     rhs=pt[:, 512:1024], start=first, stop=last)

                pending = []
                for jt in range(16):
                    sp = sps.tile([128, 1024], F32, tag="sp",
                                  name=f"sp_{m}_{iq}_{jt}")
                    # two heads on separate row tiles + separate banks
                    nc.tensor.matmul(sp[:, 0:512],
                                     lhsT=kT_sb[0:64, m, jt * 128:(jt + 1) * 128],
                                     rhs=qT_sb[0:64, m, i0:i0 + IQ],
                                     start=True, stop=True)
                    nc.tensor.matmul(sp[:, 512:1024],
                                     lhsT=kT_sb[64:128, m, jt * 128:(jt + 1) * 128],
                                     rhs=qT_sb[64:128, m, i0:i0 + IQ],
                                     start=True, stop=True)
                    pt = ptp.tile([128, 1024], BF16, tag="pt",
                                  name=f"pt_{m}_{iq}_{jt}")
                    nc.scalar.activation(pt[:], sp[:], Exp)
                    idx0 = T0 + jt * 128
                    nc.vector.tensor_mul(pt[:, 0:512], pt[:, 0:512],
                                         eb_sb[:, hA, idx0 - i0:idx0 - i0 - IQ:-1])
                    nc.vector.tensor_mul(pt[:, 512:1024], pt[:, 512:1024],
                                         eb_sb[:, hB, idx0 - i0:idx0 - i0 - IQ:-1])
                    pending.append((jt, pt))
                    if len(pending) > LAG:
                        emit_pv(*pending.pop(0))
                    if bg_fast:
                        bg_fast.pop(0)(bgp)
                    elif jt % 2 == 1 and bg_slow:
                        bg_slow.pop(0)(bgp)
                for args in pending:
                    emit_pv(*args)

                # normalization: both heads' denominators sit in pv row 64
                pvf = nrm.tile([65, 1024], F32, tag="pvf", name=f"pvf_{m}_{iq}")
                nc.vector.tensor_copy(pvf[:], pv[:])
                rsq = nrm.tile([128, 8], F32, tag="rsq", name=f"rsq_{m}_{iq}")
                nc.sync.dma_start(rsq[:], pvf[64:65, :])
                rsr = nrm.tile([128, 8], F32, tag="rsr", name=f"rsr_{m}_{iq}")
                nc.vector.reciprocal(rsr[:], rsq[:])
                rsf = nrm.tile([1, 1024], F32, tag="rsf", name=f"rsf_{m}_{iq}")
                nc.sync.dma_start(rsf[:], rsr[:])
                rsb = nrm.tile([64, 1024], F32, tag="rsb", name=f"rsb_{m}_{iq}")
                nc.gpsimd.partition_broadcast(rsb[:], rsf[:], channels=64)
                nc.vector.tensor_mul(o2_sb[0:64, m, i0:i0 + IQ],
                                     pvf[0:64, 0:512], rsb[:, 0:512])
                otmp = nrm.tile([64, 512], BF16, tag="otmp", name=f"otmp_{m}_{iq}")
                nc.vector.tensor_mul(otmp[:], pvf[0:64, 512:1024],
                                     rsb[:, 512:1024])
                nc.sync.dma_start(o2_sb[64:128, m, i0:i0 + IQ], otmp[:])

    # ---- Phase 3: output projection (partial; host sums head-groups) ----
    # yt batches both mt halves -> one DMA per ic, issued off the sync
    # queue (alternating scalar/vector) to dodge per-issue serialization.
    with tc.tile_pool(name="yps", bufs=4, space="PSUM") as yps, \
         tc.tile_pool(name="yo", bufs=3) as yo:
        for ic in range(16):
            yt = yo.tile([128, 1024], BF16, tag="yt", name=f"yt_{ic}")
            for mt in range(2):
                ps = yps.tile([128, 512], F32, tag="y", name=f"yps_{ic}_{mt}")
                for m in range(2):
                    nc.tensor.matmul(ps[:], lhsT=o2_sb[:, m, ic * 128:(ic + 1) * 128],
                                     rhs=wo_sb[:, m, mt * 512:(mt + 1) * 512],
                                     start=(m == 0), stop=(m == 1))
                if mt == 0:
                    nc.scalar.copy(yt[:, 0:512], ps[:])
                else:
                    nc.vector.tensor_copy(yt[:, 512:1024], ps[:])
            eng = nc.scalar if ic % 2 == 0 else nc.sync
            eng.dma_start(y_d[ic * 128:(ic + 1) * 128, :], yt[:])


_NC = None


def build_nc():
    global _NC
    if _NC is not None:
        return _NC
    nc = bacc.Bacc("TRN2", target_bir_lowering=False, debug=False, num_devices=8)
    xt_d = nc.dram_tensor("xt", [128, KC, T], BF16, kind="ExternalInput").ap()
    ct_d = nc.dram_tensor("ct", [128, KC, T], BF16, kind="ExternalInput").ap()
    wq_d = nc.dram_tensor("wq", [128, KC, 256], BF16, kind="ExternalInput").ap()
    wk_d = nc.dram_tensor("wk", [128, KC, 256], BF16, kind="ExternalInput").ap()
    wv_d = nc.dram_tensor("wv", [128, KC, 256], BF16, kind="ExternalInput").ap()
    wo_d = nc.dram_tensor("wo", [128, 2, D], BF16, kind="ExternalInput").ap()
    eb_d = nc.dram_tensor("eb", [128, HL, EBL], BF16, kind="ExternalInput").ap()
    y_d = nc.dram_tensor("y", [T, D], BF16, kind="ExternalOutput").ap()

    with tile.TileContext(nc) as tc, ExitStack() as ctx:
        _build_kernel(ctx, tc, y_d, xt_d, ct_d, wq_d, wk_d, wv_d, wo_d, eb_d)
    nc.compile()
    _NC = nc
    return nc


def _to_chunked(mat_t, cols, dtype):
    """[D, cols] -> [128, KC, cols] with partition dim first."""
    return np.ascontiguousarray(
        mat_t.reshape(KC, 128, cols).transpose(1, 0, 2)).astype(dtype)


def make_in_maps(x, context, Wq, Wk, Wv, Wo):
    scale = np.float32(1.0 / np.sqrt(DH))
    p = np.arange(128, dtype=np.int64)[:, None]
    t = np.arange(EBL, dtype=np.int64)[None, :]
    dist = np.abs(p + t - T0) // PERIOD          # [128, EBL]
    ws = np.float32(WS)
    in_maps = []
    for c in range(8):
        b = c // 4
        h0 = (c % 4) * HL
        rows = slice(h0 * DH, (h0 + HL) * DH)
        ct_t = np.ascontiguousarray(
            context[b].T.reshape(KC, 128, T).transpose(1, 0, 2))
        xt = np.ascontiguousarray(
            x[b].T.reshape(KC, 128, T).transpose(1, 0, 2)).astype(bf16)
        ct = ct_t.astype(bf16)
        wq = _to_chunked(np.ascontiguousarray((Wq[rows] * scale).T), 256, bf16)
        wk = _to_chunked(np.ascontiguousarray(Wk[rows].T), 256, bf16)
        wv = _to_chunked(np.ascontiguousarray(Wv[rows].T), 256, bf16)
        wo = np.ascontiguousarray(
            Wo[:, rows].T.reshape(2, 128, D).transpose(1, 0, 2)).astype(bf16)
        eb = np.empty((128, HL, EBL), dtype=bf16)
        for hl in range(HL):
            hs = 2.0 ** (-(h0 + hl + 1))
            eb[:, hl, :] = np.exp(-hs * dist).astype(bf16)
        in_maps.append({"xt": xt, "ct": ct, "wq": wq, "wk": wk,
                        "wv": wv, "wo": wo, "eb": np.ascontiguousarray(eb)})
    return in_maps


def kernel(x, context, Wq, Wk, Wv, Wo, bo, _collect=None):
    x = np.asarray(x, dtype=np.float32)
    context = np.asarray(context, dtype=np.float32)
    Wq = np.asarray(Wq, dtype=np.float32)
    Wk = np.asarray(Wk, dtype=np.float32)
    Wv = np.asarray(Wv, dtype=np.float32)
    Wo = np.asarray(Wo, dtype=np.float32)
    bo = np.asarray(bo, dtype=np.float32)

    nc = build_nc()
    in_maps = make_in_maps(x, context, Wq, Wk, Wv, Wo)
    res = run_bass_kernel_spmd(nc, in_maps, list(range(8)))
    if _collect is not None:
        _collect.append(res)

    out = np.empty((B, T, D), dtype=np.float32)
    for b in range(2):
        acc = res.results[4 * b]["y"].astype(np.float32)
        for c in range(4 * b + 1, 4 * b + 4):
            acc = acc + res.results[c]["y"].astype(np.float32)
        out[b] = acc + bo[None, :]
    return out
